# revision 1
# baseline (speedup 1.0000x reference)
"""Trainium2 Bass kernel for a dense transformer block (B=4, T=2048, C=1024, H=16).

Sharding: data-parallel over tokens. Core i owns batch b=i//2, token-half i%2
(1024 tokens). Each core redundantly computes LN1/K/V for its batch's full 2048
tokens so there are no collectives.

v2 (fp8): QKV / attention-AV / proj matmuls run in fp8e4m3 with
MatmulPerfMode.DoubleRow (contract 256 partitions per instruction = 2x bf16
throughput; measured 216ns per 512-col matmul either way on HW). Scores and
the MLP stay bf16: fp8 scores gain nothing (the 64-feature contraction is
padding-bound either way) and fp8 MLP pushes rel-err to ~2.2e-2 > the 2e-2
gate (measured via host-side quantization sim).

Scaling: fp8 weights are stored x32 (their entries are ~N(0,1/1024), fp8
subnormal range). k/q keep the x32 (folded into the exp scale C^-0.5/1024);
v is descaled by 1/32 in its epilogue; the attention denominator rides AV as
a (1/32)-column of V so o comes out x32, matching the x32 proj weights; the
proj epilogue multiplies by 2^-10.

Engine budget: ACT does exp (~253us/core, the attention-phase bound), the LN
stats' ln/exp (1/sd = exp(-.5*ln(var+eps)) keeps ACT on the one
natural_log_exp table - no 1.3us table reloads) and phase-1 PSUM->SBUF
epilogues; DVE does LN applies, o-normalize, residual adds; gpsimd does
f32->bf16 casts, broadcasts, memsets. MLP1 of query-chunk 0 is emitted
interleaved into chunk 1's attention head loop so the PE fills the gaps the
ACT-bound exp stream leaves.

PSUM: sc4 [128,4,512] (scores, 4 banks) + avp [128,512] (AV accum) + mm
[128,512] + st [128,2,512] (LN stats / spare accums) = 8 banks.
"""

import sys

if "/opt/trn_rl_repo" not in sys.path:
    sys.path.insert(0, "/opt/trn_rl_repo")

import numpy as np
import ml_dtypes

B, T, C, H, HD = 4, 2048, 1024, 16, 64
FF = 4 * C
TO = T // 2          # tokens owned per core
NCC = C // 128       # 8
NFC = FF // 128      # 32
EPS = 1e-5
SCALE = C ** -0.5    # 1/32
ESCALE = SCALE / 1024.0   # exp scale with k,q carried x32
BF16 = ml_dtypes.bfloat16
F8NP = ml_dtypes.float8_e4m3

_BUILT = None


def _emit(nc, tc, aps, has_bv):
    from concourse import mybir
    from concourse.bass import ts
    F32 = mybir.dt.float32
    BF = mybir.dt.bfloat16
    F8 = mybir.dt.float8e4
    AF = mybir.ActivationFunctionType
    PM = mybir.MatmulPerfMode
    ADD = mybir.AluOpType.add
    MULT = mybir.AluOpType.mult
    MAX = mybir.AluOpType.max
    from contextlib import ExitStack

    xT, wq, wk, wv, wproj, w1, w2, x2d, outT = (
        aps["xT"], aps["wq"], aps["wk"], aps["wv"], aps["wproj"], aps["w1"],
        aps["w2"], aps["x2d"], aps["outT"])

    ctx = ExitStack()
    with ctx:
        const = ctx.enter_context(tc.tile_pool(name="const", bufs=1))
        misc = ctx.enter_context(tc.tile_pool(name="misc", bufs=2))
        psum = ctx.enter_context(tc.tile_pool(name="psum", bufs=1, space="PSUM"))

        def ps_sc2():
            return psum.tile([128, 2, 512], F32, name="ps_sc2", tag="sc2", bufs=2)

        def ps_st():
            return psum.tile([128, 2, 512], F32, name="ps_st", tag="st", bufs=1)

        def ps_av():
            return psum.tile([128, 512], F32, name="ps_av", tag="avp", bufs=2)

        def rot6_gen():
            """6 rotating [128,512] accumulators: 2x sc2 banks + 2x avp.
            For phases where attention PSUM is free."""
            while True:
                t = ps_sc2()
                yield t[:, 0, :]
                yield t[:, 1, :]
                yield ps_av()

        def rot2_gen():
            """2 rotating accumulators clear of sc2/avp (for MLP1 interleaved
            into the attention head loop)."""
            while True:
                t = ps_st()
                yield t[:, 0, :]
                yield t[:, 1, :]

        _rot5 = rot6_gen()
        _rot3 = rot2_gen()

        # constants / biases
        ones_sc = const.tile([128, 128], BF, name="ones_sc")
        nc.vector.memset(ones_sc, 1.0 / C)
        eps_sb = const.tile([128, 1], F32, name="eps_sb")
        nc.vector.memset(eps_sb, EPS)
        bq_sb = const.tile([128, 8], F32, name="bq_sb")
        bk_sb = const.tile([128, 8], F32, name="bk_sb")
        bp_sb = const.tile([128, 8], F32, name="bp_sb")
        b2_sb = const.tile([128, 8], F32, name="b2_sb")
        b1_sb = const.tile([128, 32], F32, name="b1_sb")
        nc.sync.dma_start(out=bq_sb, in_=aps["bq"])
        nc.sync.dma_start(out=bk_sb, in_=aps["bk"])
        nc.sync.dma_start(out=bp_sb, in_=aps["bp"])
        nc.sync.dma_start(out=b2_sb, in_=aps["b2c"])
        nc.sync.dma_start(out=b1_sb, in_=aps["b1c"])
        if has_bv:
            bv_sb = const.tile([1, 1024], F32, name="bv_sb")
            nc.sync.dma_start(out=bv_sb, in_=aps["bvrow"])
            bvb = const.tile([128, 1024], F32, name="bvb")
            nc.gpsimd.partition_broadcast(bvb, bv_sb)

        def stats_finish(st, tag, bufs=4):
            """st: psum [:,0]=mean, [:,1]=E[x^2] (rows replicated).
            1/sd via exp(-0.5*ln(var+eps)) so ACT stays on the exp table."""
            mu_sb = misc.tile([128, 512], F32, name="mu_sb", tag=tag, bufs=bufs)
            nc.vector.tensor_copy(out=mu_sb, in_=st[:, 0, :])
            musq = misc.tile([128, 512], F32, name="musq", tag="stat", bufs=2)
            nc.vector.tensor_mul(out=musq, in0=mu_sb, in1=mu_sb)
            var = misc.tile([128, 512], F32, name="var", tag="stat", bufs=2)
            nc.vector.tensor_sub(out=var, in0=st[:, 1, :], in1=musq)
            lnv = misc.tile([128, 512], F32, name="lnv", tag="stat", bufs=2)
            nc.scalar.activation(out=lnv, in_=var, func=AF.Ln, bias=eps_sb)
            s_sb = misc.tile([128, 512], F32, name="s_sb", tag=tag, bufs=bufs)
            nc.scalar.activation(out=s_sb, in_=lnv, func=AF.Exp, scale=-0.5)
            return mu_sb, s_sb

        def ln_chunk(src_dram, tcg, h_dst):
            """LN of one 512-token chunk of src_dram [8,128,ntok] into h_dst
            ([128,8,512] fp8). Stats from transient bf16 copies; apply
            re-reads the f32 source so no 8-deep bf16 buffer is held."""
            st = ps_st()
            for c in range(NCC):
                xs = misc.tile([128, 512], F32, name="xs", tag="xs", bufs=4)
                nc.sync.dma_start(out=xs, in_=src_dram[c, :, ts(tcg, 512)])
                xbf = misc.tile([128, 512], BF, name="xbf", tag="bfts", bufs=3)
                nc.scalar.copy(out=xbf, in_=xs)
                xsq = misc.tile([128, 512], BF, name="xsq", tag="bfts", bufs=3)
                nc.vector.tensor_mul(out=xsq, in0=xbf, in1=xbf)
                nc.tensor.matmul(st[:, 0, :], ones_sc, xbf,
                                 start=(c == 0), stop=(c == NCC - 1),
                                 skip_group_check=True)
                nc.tensor.matmul(st[:, 1, :], ones_sc, xsq,
                                 start=(c == 0), stop=(c == NCC - 1),
                                 skip_group_check=True)
            mu_sb, s_sb = stats_finish(st, "mstat", bufs=3)
            for c in range(NCC):
                xs = misc.tile([128, 512], F32, name="xs", tag="xs", bufs=4)
                nc.sync.dma_start(out=xs, in_=src_dram[c, :, ts(tcg, 512)])
                d = misc.tile([128, 512], F32, name="d", tag="xs", bufs=4)
                nc.vector.tensor_sub(out=d, in0=xs, in1=mu_sb)
                nc.vector.tensor_mul(out=h_dst[:, c, :], in0=d, in1=s_sb)

        ln2_stats = {}
        h2_tiles = {}
        z_tiles = {}

        def ln2_apply(qc, mlpp):
            mu2, s2 = ln2_stats[qc]
            h2 = mlpp.tile([128, 8, 512], BF, name="h2", tag="h2", bufs=1)
            for c in range(NCC):
                xs2 = misc.tile([128, 512], F32, name="xs2", tag="xs", bufs=4)
                nc.sync.dma_start(out=xs2, in_=x2d[c, :, ts(qc, 512)])
                d2 = misc.tile([128, 512], F32, name="d2", tag="xs", bufs=4)
                nc.vector.tensor_sub(out=d2, in0=xs2, in1=mu2)
                nc.vector.tensor_mul(out=h2[:, c, :], in0=d2, in1=s2)
            h2_tiles[qc] = h2
            z_tiles[qc] = mlpp.tile([128, 32, 512], BF, name="z_sb",
                                    tag="z", bufs=1)

        def mlp1_block(qc, mg, rot, relu_dve, w1pool):
            """One mg (4 m-tiles of 128) of MLP1 for query chunk qc."""
            h2 = h2_tiles[qc]
            z_sb = z_tiles[qc]
            w1t = [w1pool.tile([128, 512], BF, name="w1t", tag="w1s")
                   for _ in range(NCC)]
            for k in range(NCC):
                nc.sync.dma_start(out=w1t[k], in_=w1[mg, k])
            for m4 in range(4):
                acc = next(rot)
                for k in range(NCC):
                    nc.tensor.matmul(acc, w1t[k][:, ts(m4, 128)],
                                     h2[:, k, :], start=(k == 0),
                                     stop=(k == NCC - 1),
                                     skip_group_check=True)
                m = mg * 4 + m4
                if relu_dve:
                    nc.vector.tensor_scalar(
                        out=z_sb[:, m, :], in0=acc,
                        scalar1=b1_sb[:, m:m + 1], scalar2=0.0,
                        op0=ADD, op1=MAX)
                else:
                    nc.scalar.activation(out=z_sb[:, m, :], in_=acc,
                                         func=AF.Relu,
                                         bias=b1_sb[:, m:m + 1])

        with tc.tile_pool(name="mlp", bufs=1) as mlpp:
            # ============ scope A: LN1 + QKV + attention + proj ============
            with tc.tile_pool(name="attn", bufs=1) as attn:
                k_sb = attn.tile([128, 8, T], BF, name="k_sb")
                qz0 = attn.tile([128, 8, TO], BF, name="qz0")
                qz1 = attn.tile([128, 8, TO], BF, name="qz1")
                o_sb = attn.tile([128, 8, TO], F8, name="o_sb")
                v_aug = attn.tile([128, 16, 16, 128], F8, name="v_aug")
                nc.gpsimd.memset(qz0[64:128, :, :], 0.0)
                nc.gpsimd.memset(qz1[0:64, :, :], 0.0)
                # denominator col 64 = 1/32; pad cols 65.. stay uninitialized
                # (their AV psum rows are never read)
                nc.vector.memset(v_aug[:, :, :, 64:65], 1.0 / 32.0)

                # ---- phase 1: LN1 chunks + K and V projections ----
                with tc.tile_pool(name="lnp", bufs=4) as lnp:
                  with tc.tile_pool(name="wkv", bufs=8) as wkv:
                    hts = []
                    wtk = wtv = None
                    for tcg in range(4):
                        h_t = lnp.tile([128, 8, 512], F8, name="h_t", tag="h")
                        ln_chunk(xT, tcg, h_t)
                        hts.append(h_t)
                        if wtk is None:
                            wtk = [wkv.tile([128, 2, 1024], F8, name="wtk",
                                            tag="wkvt") for _ in range(4)]
                            wtv = [wkv.tile([128, 2, 1024], F8, name="wtv",
                                            tag="wkvt") for _ in range(4)]
                            for p in range(4):
                                nc.sync.dma_start(out=wtk[p], in_=wk[p])
                                nc.sync.dma_start(out=wtv[p], in_=wv[p])
                        # K: out k' = 32k [128 feats, 512 tok]
                        for m in range(NCC):
                            kp = next(_rot5)
                            for p in range(4):
                                nc.tensor.matmul(kp, wtk[p][:, :, ts(m, 128)],
                                                 h_t[:, 2 * p:2 * p + 2, :],
                                                 start=(p == 0), stop=(p == 3),
                                                 perf_mode=PM.DoubleRow,
                                                 skip_group_check=True)
                            nc.scalar.activation(
                                out=k_sb[:, m, ts(tcg, 512)], in_=kp,
                                func=AF.Identity, bias=bk_sb[:, m:m + 1])
                        # V: out [128 tok, 512 feats]; epilogue x(1/32) -> fp8
                        for tt in range(4):
                            for nch in range(2):
                                vp = next(_rot5)
                                for p in range(4):
                                    nc.tensor.matmul(
                                        vp, h_t[:, 2 * p:2 * p + 2,
                                                ts(tt, 128)],
                                        wtv[p][:, :, ts(nch, 512)],
                                        start=(p == 0), stop=(p == 3),
                                        perf_mode=PM.DoubleRow,
                                        skip_group_check=True)
                                dst = v_aug[:, tcg * 4 + tt,
                                            nch * 8:(nch + 1) * 8, 0:64]
                                src = vp.rearrange("p (h d) -> p h d", h=8)
                                if has_bv:
                                    vt = misc.tile([128, 8, 64], F32,
                                                   name="vt", tag="vt", bufs=2)
                                    bslice = bvb[:, ts(nch, 512)].rearrange(
                                        "p (h d) -> p h d", h=8)
                                    nc.vector.tensor_add(out=vt, in0=src,
                                                         in1=bslice)
                                    nc.scalar.mul(dst, vt, 1.0 / 32.0)
                                else:
                                    nc.scalar.mul(dst, src, 1.0 / 32.0)
                  # Q projection (own tokens only): q' = 32q
                  with tc.tile_pool(name="wqp", bufs=4) as wqp:
                      wtq = [wqp.tile([128, 2, 1024], F8, name="wtq",
                                      tag="wq") for _ in range(4)]
                      for p in range(4):
                          nc.sync.dma_start(out=wtq[p], in_=wq[p])
                      for tcg in range(2):
                          for m in range(NCC):
                              qp = next(_rot5)
                              for p in range(4):
                                  nc.tensor.matmul(
                                      qp, wtq[p][:, :, ts(m, 128)],
                                      hts[tcg][:, 2 * p:2 * p + 2, :],
                                      start=(p == 0), stop=(p == 3),
                                      perf_mode=PM.DoubleRow,
                                      skip_group_check=True)
                              nc.scalar.activation(
                                  out=qz0[0:64, m, ts(tcg, 512)],
                                  in_=qp[0:64, :],
                                  func=AF.Identity,
                                  bias=bq_sb[0:64, m:m + 1])
                              nc.scalar.activation(
                                  out=qz1[64:128, m, ts(tcg, 512)],
                                  in_=qp[64:128, :],
                                  func=AF.Identity,
                                  bias=bq_sb[64:128, m:m + 1])

                # ---- attention + proj per query chunk; MLP1(qc0) rides
                #      inside qc1's head loop ----
                with tc.tile_pool(name="wpp", bufs=4) as wpp, \
                     tc.tile_pool(name="expp", bufs=2) as expp, \
                     tc.tile_pool(name="stage", bufs=2) as stage, \
                     tc.tile_pool(name="w1p", bufs=8) as w1p:
                    wtp = [wpp.tile([128, 2, 1024], F8, name="wtp", tag="wp")
                           for _ in range(4)]
                    for p in range(4):
                        nc.sync.dma_start(out=wtp[p], in_=wproj[p])

                    for qc in range(2):
                        for h in range(H):
                            hp = h // 2
                            qz = qz0 if h % 2 == 0 else qz1
                            avp = ps_av()
                            for g in range(8):
                                scp = ps_sc2()
                                for j in range(2):
                                    nc.tensor.matmul(
                                        scp[:, j, :],
                                        k_sb[:, hp, ts(2 * g + j, 128)],
                                        qz[:, hp, ts(qc, 512)],
                                        start=True, stop=True)
                                et = expp.tile([128, 2, 512], F8, name="et",
                                               tag="et")
                                nc.scalar.activation(out=et, in_=scp,
                                                     func=AF.Exp,
                                                     scale=ESCALE)
                                nc.tensor.matmul(
                                    avp, v_aug[:, 2 * g:2 * g + 2, h, :],
                                    et, start=(g == 0), stop=(g == 7),
                                    perf_mode=PM.DoubleRow,
                                    skip_group_check=True)
                            r_t = misc.tile([1, 512], F32, name="r_t",
                                            tag="r", bufs=2)
                            nc.vector.reciprocal(out=r_t, in_=avp[64:65, :])
                            rb_t = misc.tile([64, 512], F32, name="rb_t",
                                             tag="rb", bufs=2)
                            nc.gpsimd.partition_broadcast(rb_t, r_t)
                            p0 = (h % 2) * 64
                            nc.vector.tensor_mul(
                                out=o_sb[p0:p0 + 64, hp, ts(qc, 512)],
                                in0=avp[0:64, :], in1=rb_t)
                            # interleave MLP1(qc0) into qc1's head loop
                            if qc == 1 and h % 2 == 0:
                                mlp1_block(0, h // 2, _rot3, True, w1p)
                        # proj + residual -> x2 (DRAM), LN2 stats fused
                        st2 = ps_st()
                        for m in range(NCC):
                            pp = ps_av()
                            for p in range(4):
                                nc.tensor.matmul(
                                    pp, wtp[p][:, :, ts(m, 128)],
                                    o_sb[:, 2 * p:2 * p + 2, ts(qc, 512)],
                                    start=(p == 0), stop=(p == 3),
                                    perf_mode=PM.DoubleRow,
                                    skip_group_check=True)
                            xres = stage.tile([128, 512], F32, name="xres",
                                              tag="xres")
                            nc.sync.dma_start(out=xres,
                                              in_=xT[m, :, ts(qc, 512)])
                            x2t = stage.tile([128, 512], F32, name="x2t",
                                             tag="x2t")
                            nc.vector.scalar_tensor_tensor(
                                out=x2t, in0=pp, scalar=1.0 / 1024.0,
                                in1=xres, op0=MULT, op1=ADD)
                            nc.sync.dma_start(out=x2d[m, :, ts(qc, 512)],
                                              in_=x2t)
                            xb2 = stage.tile([128, 512], BF, name="xb2",
                                             tag="xb2")
                            nc.scalar.copy(out=xb2, in_=x2t)
                            xq2 = misc.tile([128, 512], BF, name="xq2",
                                            tag="bfts", bufs=3)
                            nc.vector.tensor_mul(out=xq2, in0=xb2, in1=xb2)
                            nc.tensor.matmul(st2[:, 0, :], ones_sc, xb2,
                                             start=(m == 0),
                                             stop=(m == NCC - 1),
                                             skip_group_check=True)
                            nc.tensor.matmul(st2[:, 1, :], ones_sc, xq2,
                                             start=(m == 0),
                                             stop=(m == NCC - 1),
                                             skip_group_check=True)
                        ln2_stats[qc] = stats_finish(st2, "mstat", bufs=3)
                        if qc == 0:
                            ln2_apply(0, mlpp)

            # ============ scope B: MLP2(qc0), LN2+MLP1+MLP2 for qc1 ========
            def acc8():
                """All 8 psum banks as [128,512] accumulator views."""
                a, b, st = ps_sc2(), ps_sc2(), ps_st()
                return ([a[:, j, :] for j in range(2)]
                        + [b[:, j, :] for j in range(2)]
                        + [st[:, j, :] for j in range(2)]
                        + [ps_av(), ps_av()])

            with tc.tile_pool(name="w2p", bufs=8) as w2p, \
                 tc.tile_pool(name="w1pb", bufs=16) as w1pb, \
                 tc.tile_pool(name="outp", bufs=3) as outp:

                def mlp2(qc):
                    z_sb = z_tiles[qc]
                    accs = acc8()
                    for k in range(NFC):
                        w2t = w2p.tile([128, 1024], BF, name="w2t", tag="w2s")
                        nc.sync.dma_start(out=w2t, in_=w2[k])
                        for m in range(NCC):
                            nc.tensor.matmul(accs[m], w2t[:, ts(m, 128)],
                                             z_sb[:, k, :], start=(k == 0),
                                             stop=(k == NFC - 1),
                                             skip_group_check=True)
                    for m in range(NCC):
                        xr2 = outp.tile([128, 512], F32, name="xr2",
                                        tag="xr2")
                        nc.sync.dma_start(out=xr2, in_=x2d[m, :, ts(qc, 512)])
                        ot = outp.tile([128, 512], F32, name="ot", tag="ot")
                        nc.vector.scalar_tensor_tensor(
                            out=ot, in0=accs[m], scalar=b2_sb[:, m:m + 1],
                            in1=xr2, op0=ADD, op1=ADD)
                        nc.sync.dma_start(out=outT[m, :, ts(qc, 512)], in_=ot)

                mlp2(0)
                ln2_apply(1, mlpp)
                for mg in range(8):
                    mlp1_block(1, mg, _rot5, False, w1pb)
                mlp2(1)


def _build(has_bv):
    from concourse import bacc, mybir, tile
    F32 = mybir.dt.float32
    BF = mybir.dt.bfloat16
    F8 = mybir.dt.float8e4

    nc = bacc.Bacc("TRN2", target_bir_lowering=False, debug=False,
                   enable_asserts=False, num_devices=8)
    aps = {}
    aps["xT"] = nc.dram_tensor("xT", [8, 128, T], F32, kind="ExternalInput").ap()
    for n in ("wq", "wk", "wv", "wproj"):
        aps[n] = nc.dram_tensor(n, [4, 128, 2, 1024], F8,
                                kind="ExternalInput").ap()
    aps["w1"] = nc.dram_tensor("w1", [8, 8, 128, 512], BF,
                               kind="ExternalInput").ap()
    aps["w2"] = nc.dram_tensor("w2", [32, 128, 1024], BF,
                               kind="ExternalInput").ap()
    for n in ("bq", "bk", "bp", "b2c"):
        aps[n] = nc.dram_tensor(n, [128, 8], F32, kind="ExternalInput").ap()
    aps["b1c"] = nc.dram_tensor("b1c", [128, 32], F32, kind="ExternalInput").ap()
    if has_bv:
        aps["bvrow"] = nc.dram_tensor("bvrow", [1, 1024], F32,
                                      kind="ExternalInput").ap()
    aps["x2d"] = nc.dram_tensor("x2d", [8, 128, TO], F32).ap()
    aps["outT"] = nc.dram_tensor("outT", [8, 128, TO], F32,
                                 kind="ExternalOutput").ap()

    with tile.TileContext(nc) as tcx:
        _emit(nc, tcx, aps, has_bv)
    nc.compile()
    return nc


def _prep_inputs(x, Wq, Wk, Wv, Wproj, bproj, W1, b1, W2, b2, g1, be1, g2, be2):
    """Host-side prep: fold LN affine into weights/biases, cast, lay out."""
    x = np.asarray(x, np.float32)
    g1 = np.asarray(g1, np.float32)
    be1 = np.asarray(be1, np.float32)
    g2 = np.asarray(g2, np.float32)
    be2 = np.asarray(be2, np.float32)

    def to2d(w):  # (H, C, hd) -> (C, H*hd)
        return np.asarray(w, np.float32).transpose(1, 0, 2).reshape(C, C)

    wq2, wk2, wv2 = to2d(Wq), to2d(Wk), to2d(Wv)
    Wproj = np.asarray(Wproj, np.float32)
    W1 = np.asarray(W1, np.float32)
    W2 = np.asarray(W2, np.float32)

    wq_e, wk_e, wv_e = g1[:, None] * wq2, g1[:, None] * wk2, g1[:, None] * wv2
    w1_e = g2[:, None] * W1
    bias_q = 32.0 * (be1 @ wq2)   # k', q' carry x32
    bias_k = 32.0 * (be1 @ wk2)
    bias_v = be1 @ wv2            # v is descaled in its epilogue
    bias_1 = np.asarray(b1, np.float32) + be2 @ W1

    def wpair8(w):  # (C, N) -> (4, 128, 2, N) fp8, x32
        a = (32.0 * w).reshape(4, 2, 128, -1).transpose(0, 2, 1, 3)
        return np.ascontiguousarray(a.astype(F8NP))

    def bvec(v):  # (N,) -> (128, N//128) partition-major
        return np.ascontiguousarray(np.asarray(v, np.float32).reshape(-1, 128).T)

    shared = {
        "wq": wpair8(wq_e), "wk": wpair8(wk_e), "wv": wpair8(wv_e),
        "wproj": wpair8(Wproj),
        "w1": np.ascontiguousarray(
            w1_e.reshape(NCC, 128, 8, 512).transpose(2, 0, 1, 3).astype(BF16)),
        "w2": np.ascontiguousarray(W2.reshape(NFC, 128, C).astype(BF16)),
        "bq": bvec(bias_q), "bk": bvec(bias_k),
        "bp": bvec(np.asarray(bproj, np.float32)),
        "b2c": bvec(np.asarray(b2, np.float32)), "b1c": bvec(bias_1),
    }
    has_bv = bool(np.any(bias_v != 0.0))
    if has_bv:
        shared["bvrow"] = np.ascontiguousarray(bias_v.reshape(1, C))

    in_maps = []
    for core in range(8):
        b, half = core // 2, core % 2
        xt = x[b].T  # (C, T)
        own = xt[:, half * TO:(half + 1) * TO]
        oth = xt[:, (1 - half) * TO:(2 - half) * TO]
        m = dict(shared)
        m["xT"] = np.ascontiguousarray(
            np.concatenate([own, oth], axis=1).reshape(NCC, 128, T))
        in_maps.append(m)
    return in_maps, has_bv


def kernel(x, Wq, Wk, Wv, Wproj, bproj, W1, b1, W2, b2, g1, be1, g2, be2):
    global _BUILT
    from concourse.bass_utils import run_bass_kernel_spmd

    in_maps, has_bv = _prep_inputs(x, Wq, Wk, Wv, Wproj, bproj, W1, b1, W2, b2,
                                   g1, be1, g2, be2)
    if _BUILT is None or _BUILT[1] != has_bv:
        _BUILT = (_build(has_bv), has_bv)
    nc = _BUILT[0]
    res = run_bass_kernel_spmd(nc, in_maps, core_ids=list(range(8)))
    out = np.empty((B, T, C), np.float32)
    for core in range(8):
        b, half = core // 2, core % 2
        o = res.results[core]["outT"].reshape(C, TO)  # (feature, token)
        out[b, half * TO:(half + 1) * TO, :] = o.T
    return out

